# revision 76
# baseline (speedup 1.0000x reference)
"""Multi-head causal attention (B=4, S=2048, D=1024, H=16) on 8 trn2 cores.

Sharding: tensor-parallel over heads x data-parallel over batch.
core c -> (batch b = c//2, head-group hg = c%2 of 8 heads). Every core runs
an identical SPMD program on different data:
  - QKV projections (bf16 weights/activations, f32 PSUM accumulate) for its
    512 features. K kept transposed [feat, seq] in SBUF; V kept [seq, feat]
    with an appended ones column per head (softmax denominators come free
    out of the PV matmul); Q written per-head into zero-padded [128, 512]
    tiles (other head's partitions zeroed) so scores run as a single K=128
    N=512 matmul against the two-head-stacked K tile.
  - Causal attention per (head, superblock), two 128-key blocks per
    "exp-pair": both score matmuls write one 2-bank PSUM tile, ONE ScalarE
    exp covers both (amortizes the ~185ns access latency; no max
    subtraction - scores are O(5) so exp cannot overflow). Diagonal blocks
    are causally trimmed to their valid query ranges (512/384/256/128) and
    masked with prebuilt pair masks in one DVE multiply (in-place, 2-byte
    2x mode). PV accumulates in PSUM with an M=65 stationary (64 V dims +
    ones). History pairs are emitted before diagonal pairs so exp-bound and
    matmul-bound stretches interleave across heads.
  - Normalization: denominator row copied to SBUF (ScalarE), raw ctx copied
    out of PSUM early to release the accumulator, GpSimd partition
    broadcast, DVE fast reciprocal + scale; ctx^T staged directly in SBUF
    tiles (no DRAM round trip) that the output projection reads.
  - Output projection (bf16) against the head-group's 512-column slice of
    Wo, bf16 partials DMA'd out.
Host sums the two partial outputs per batch in f32 (the "all-reduce after
W_o" done at gather time) and folds the Wo @ bv + bo constant.

Scheduling: projections for superblock sc+1 and the output projection for
sc-1 are interleaved between attention batches of sc (keeps the PE fed
where ScalarE exp would otherwise pace). At the tail, a few o-proj thunks
are held back past the last attention batch and the final head's normalize
is emitted after them (DMA-queue semaphore thresholds are cumulative, so
anything emitted after that last cfl write would wait on it).

Numerics: all matmul operands bf16 (f32 PSUM accumulation throughout);
measured rel-l2 vs the f32 reference is ~5.6e-3 (gate 2e-2). The zero
padding of Q tiles is exact in bf16, so no cross-head leakage.
"""

import sys

import numpy as np

_BASS_PATH = "/opt/trn_rl_repo"
if _BASS_PATH not in sys.path:
    sys.path.insert(0, _BASS_PATH)

B, S, D, H, DK = 4, 2048, 1024, 16, 64
NCORES = 8
FH = 512  # features per core (8 heads)
HL = 8  # local heads
NSC = 4  # seq superblocks of 512
SQ = 512
NKB = 16  # key blocks of 128
NDM = 8  # d_model chunks of 128

_cache = {}


def _round_f32r(x: np.ndarray) -> np.ndarray:
    """Round fp32 to fp32r (RNE to 11 mantissa bits) - matches TRN2 HW."""
    v = np.ascontiguousarray(x, dtype=np.float32).view(np.uint32)
    lsb = (v >> np.uint32(12)) & np.uint32(1)
    out = ((v + np.uint32(0x7FF) + lsb) >> np.uint32(12)) << np.uint32(12)
    return out.view(np.float32)


def _build():
    import concourse.bacc as bacc
    import concourse.mybir as mybir
    from concourse.tile import TileContext

    f32, f32r = mybir.dt.float32, mybir.dt.float32r
    AF = mybir.ActivationFunctionType

    nc = bacc.Bacc("TRN2", target_bir_lowering=False, debug=False, num_devices=1)

    bf16 = mybir.dt.bfloat16
    xq_d = nc.dram_tensor("xq", [D, S], bf16, kind="ExternalInput").ap()
    xk_d = nc.dram_tensor("xk", [D, S], bf16, kind="ExternalInput").ap()
    xv_d = nc.dram_tensor("xv", [D, S], bf16, kind="ExternalInput").ap()
    wq_d = nc.dram_tensor("wq", [D, FH], bf16, kind="ExternalInput").ap()
    wk_d = nc.dram_tensor("wk", [D, FH], bf16, kind="ExternalInput").ap()
    wv_d = nc.dram_tensor("wv", [D, FH], bf16, kind="ExternalInput").ap()
    wo_d = nc.dram_tensor("wo", [FH, D], bf16, kind="ExternalInput").ap()
    # pair masks for the two diagonal exp-pairs (see make_attn_batches):
    # cols 0:896 = pm0 (j0 full at 0:512, j1 trimmed to q>=128 at 512:896),
    # cols 896:1280 = pm1 (j2 q>=256 at 0:256, j3 q>=384 at 256:384)
    mask_d = nc.dram_tensor("masks", [128, 1280], bf16, kind="ExternalInput").ap()
    bq_d = nc.dram_tensor("bq", [FH], f32, kind="ExternalInput").ap()
    bk_d = nc.dram_tensor("bk", [FH], f32, kind="ExternalInput").ap()
    # zeros for the qp pad halves (memset doesn't support f32r/oddball types)
    zq_d = nc.dram_tensor("zq", [64, SQ], bf16, kind="ExternalInput").ap()
    # bf16 partials (summed in f32 on the host) — halves output DMA
    out_d = nc.dram_tensor("out", [S, D], bf16, kind="ExternalOutput").ap()

    with TileContext(nc) as tc:
        with (
            tc.tile_pool(name="res", bufs=1) as res,
            tc.tile_pool(name="st", bufs=1) as st,
            tc.tile_pool(name="psum", bufs=1, space="PSUM") as psp,
        ):
            kt = [res.tile([128, S], bf16, name=f"kt{i}", tag=f"kt{i}") for i in range(4)]
            # 520 data cols: 8 heads x (64 V dims + a ones column per head);
            # PV uses an M=65 stationary slice so no inter-head padding needed
            vaug = [
                res.tile([128, 520], bf16, name=f"va{k}", tag=f"va{k}")
                for k in range(NKB)
            ]
            pm0_t = res.tile([128, 896], bf16, name="pm0", tag="pm0")
            pm1_t = res.tile([128, 384], bf16, name="pm1", tag="pm1")
            bq_t = [res.tile([128, 1], f32, name=f"bq{i}", tag=f"bq{i}") for i in range(4)]
            bk_t = [res.tile([128, 1], f32, name=f"bk{i}", tag=f"bk{i}") for i in range(4)]
            for i in range(4):
                nc.sync.dma_start(
                    bq_t[i][:],
                    bq_d[i * 128 : (i + 1) * 128].rearrange("(p o) -> p o", o=1),
                )
                nc.sync.dma_start(
                    bk_t[i][:],
                    bk_d[i * 128 : (i + 1) * 128].rearrange("(p o) -> p o", o=1),
                )
            ones_t = res.tile([128, HL], bf16, name="ones", tag="ones")
            nc.vector.memset(ones_t[:], 1.0)
            # per-head padded Q^T tiles (double-buffered by superblock
            # parity): head h's 64 dims live in partitions (h%2)*64..+64, the
            # other 64 partitions are zero so the K=128 score matmul against
            # the stacked two-head kt tile picks out exactly head h.
            qp = [
                [
                    res.tile([128, SQ], bf16, name=f"qp{h}_{p}", tag=f"qp{h}_{p}")
                    for p in range(2)
                ]
                for h in range(HL)
            ]
            wo_sb = [
                res.tile([128, D], bf16, name=f"wo{fc}", tag=f"wo{fc}")
                for fc in range(4)
            ]

            def load_constants():
                # emitted AFTER the sb0 projection loads so the DMA queues
                # deliver proj operands first (cuts the startup stall); in
                # first-use order: qp zero-pads (scores), masks, then wo
                for h in range(HL):
                    zs = slice(64, 128) if h % 2 == 0 else slice(0, 64)
                    for p in range(2):
                        nc.sync.dma_start(qp[h][p][zs, :], zq_d[:])
                nc.sync.dma_start(pm0_t[:], mask_d[:, 0:896])
                nc.sync.dma_start(pm1_t[:], mask_d[:, 896:1280])
                for fc in range(4):
                    nc.sync.dma_start(
                        wo_sb[fc][:], wo_d[fc * 128 : (fc + 1) * 128, :]
                    )
            # ctx^T staging in SBUF, double-buffered by superblock parity;
            # o-proj for sb reads cfl[sb & 1] directly (no DRAM round trip)
            cfl = [
                [
                    res.tile([128, SQ], bf16, name=f"cfl{p}_{fc}", tag=f"cfl{p}_{fc}")
                    for fc in range(4)
                ]
                for p in range(2)
            ]

            # all three projection weight sets stay resident in SBUF (bf16
            # makes them cheap: 3 x 8KB/partition); loaded once at start in
            # first-use order, so warm thunks only ever DMA activations
            wres = {}
            for pname, w_d in (("k", wk_d), ("v", wv_d), ("q", wq_d)):
                wres[pname] = [
                    res.tile([128, FH], bf16, name=f"w{pname}{dm}", tag=f"w{pname}{dm}")
                    for dm in range(NDM)
                ]

            def load_weights(pname, w_d):
                for dm in range(NDM):
                    if pname == "k" and dm < 2:
                        # first tiles 4-way split across DMA queues: the very
                        # first matmul waits only on these, and per-queue
                        # bandwidth (not aggregate) bounds a single transfer
                        for c in range(4):
                            nc.sync.dma_start(
                                wres[pname][dm][c * 32 : (c + 1) * 32, :],
                                w_d[dm * 128 + c * 32 : dm * 128 + (c + 1) * 32, :],
                            )
                    else:
                        nc.sync.dma_start(
                            wres[pname][dm][:], w_d[dm * 128 : (dm + 1) * 128, :]
                        )

            def make_proj_thunks(sc):
                thunks = []
                for pname, x_d in (
                    ("k", xk_d),
                    ("v", xv_d),
                    ("q", xq_d),
                ):
                    box = {}

                    def load(pname=pname, x_d=x_d, box=box, sc=sc):
                        xr = []
                        for dm in range(NDM):
                            xt = st.tile(
                                [128, SQ], bf16, name=f"x{dm}", tag=f"x{dm}", bufs=3
                            )
                            if pname == "k" and sc == 0 and dm < 2:
                                for c in range(4):
                                    nc.sync.dma_start(
                                        xt[c * 32 : (c + 1) * 32, :],
                                        x_d[
                                            dm * 128 + c * 32 : dm * 128 + (c + 1) * 32,
                                            0:SQ,
                                        ],
                                    )
                            else:
                                nc.sync.dma_start(
                                    xt[:],
                                    x_d[
                                        dm * 128 : (dm + 1) * 128,
                                        sc * SQ : (sc + 1) * SQ,
                                    ],
                                )
                            xr.append(xt)
                        box["w"], box["x"] = wres[pname], xr

                    for gi in range(4):

                        def group(pname=pname, gi=gi, box=box, sc=sc, load=load):
                            if gi == 0:
                                load()
                            w_sb, xr = box["w"], box["x"]
                            if pname in ("q", "k"):
                                pp = psp.tile(
                                    [128, SQ], f32, name="pp", tag="pp", bufs=2
                                )
                                for dm in range(NDM):
                                    nc.tensor.matmul(
                                        pp[:],
                                        w_sb[dm][:, gi * 128 : (gi + 1) * 128],
                                        xr[dm][:],
                                        start=(dm == 0),
                                        stop=(dm == NDM - 1),
                                    )
                                # bias-adds on Scalar (DVE's in-order queue
                                # carries the attention-consumer ops — mask,
                                # ob casts; producer-side adds there cause
                                # priority inversion, measured 2x slowdown)
                                if pname == "k":
                                    nc.scalar.activation(
                                        kt[gi][:, sc * SQ : (sc + 1) * SQ],
                                        pp[:],
                                        AF.Identity,
                                        bias=bk_t[gi][:],
                                    )
                                else:
                                    # split the two heads of this feature
                                    # chunk into their padded qp tiles:
                                    # even head on Scalar, odd on DVE
                                    par = sc & 1
                                    nc.scalar.activation(
                                        qp[2 * gi][par][0:64, :],
                                        pp[0:64, :],
                                        AF.Identity,
                                        bias=bq_t[gi][0:64, :],
                                    )
                                    nc.vector.tensor_scalar_add(
                                        qp[2 * gi + 1][par][64:128, :],
                                        pp[64:128, :],
                                        bq_t[gi][64:128, :],
                                    )
                            else:  # v
                                kb = sc * 4 + gi
                                pp = psp.tile(
                                    [128, FH], f32, name="pp", tag="pp", bufs=2
                                )
                                for dm in range(NDM):
                                    nc.tensor.matmul(
                                        pp[:],
                                        xr[dm][:, gi * 128 : (gi + 1) * 128],
                                        w_sb[dm][:],
                                        start=(dm == 0),
                                        stop=(dm == NDM - 1),
                                    )
                                va3 = vaug[kb][:, 0 : HL * 65].rearrange(
                                    "p (h e) -> p h e", e=65
                                )
                                pp3 = pp[:].rearrange("p (h e) -> p h e", e=64)
                                nc.vector.tensor_copy(va3[:, :, 0:64], pp3[:])
                                nc.vector.tensor_copy(
                                    va3[:, :, 64:65],
                                    ones_t[:].rearrange("p (h o) -> p h o", o=1),
                                )

                        thunks.append(group)
                return thunks

            def make_attn_batches(h, sb, defer_norm=False):
                """Thunks for one (head, superblock): 2 exp-pairs per thunk.

                Each pair shares one 2-bank PSUM tile and ONE exp instruction
                (amortizes the ~185ns activation access latency). Diagonal
                blocks are causally trimmed: block j only computes queries
                >= j*128 (j=3 keeps N=256 for the f32r full-rate minimum and
                lets the mask zero queries < 384).
                visit = (kb, qoff, width, es_col)."""
                ti, po = h // 2, (h % 2) * 64
                par = sb & 1
                nkb = 4 * (sb + 1)
                d0 = sb * 4
                # history (PE-heavy) pairs first, diagonal (Scalar-heavy
                # exp+mask) pairs last: consecutive heads then interleave
                # exp-bound and matmul-bound stretches. First-emitted PV is
                # always full width (kb=0 for sb>0, j0 for sb=0), as the
                # PSUM accumulation start requires.
                pairs = [
                    ([(kb, 0, 512, 0), (kb + 1, 0, 512, 512)], 1024, None)
                    for kb in range(0, 4 * sb, 2)
                ]
                pairs += [
                    ([(d0, 0, 512, 0), (d0 + 1, 128, 384, 512)], 896, pm0_t),
                    ([(d0 + 2, 256, 256, 0), (d0 + 3, 384, 128, 256)], 384, pm1_t),
                ]
                state = {}

                def batch(pi):
                    if pi == 0:
                        state["cp"] = psp.tile(
                            [128, SQ], f32, name="cp", tag="cp", bufs=2
                        )
                        state["emitted"] = 0
                    cp = state["cp"]
                    ready = []
                    for visits, ew, pm in pairs[pi : pi + 2]:
                        sp = psp.tile([128, 1024], f32, name="sp", tag="sp", bufs=2)
                        for kb, qoff, w, ec in visits:
                            nc.tensor.matmul(
                                sp[:, ec : ec + w],
                                kt[ti][:, kb * 128 : (kb + 1) * 128],
                                qp[h][par][:, qoff : qoff + w],
                                start=True,
                                stop=True,
                            )
                        es = st.tile([128, 1024], bf16, name="es", tag="es", bufs=4)
                        nc.scalar.activation(es[:, 0:ew], sp[:, 0:ew], AF.Exp)
                        if pm is not None:
                            # mask multiply in place (saves an SBUF tag)
                            nc.vector.tensor_mul(
                                es[:, 0:ew], es[:, 0:ew], pm[:, 0:ew]
                            )
                        ready.append((visits, es))
                    for visits, es in ready:
                        for kb, qoff, w, ec in visits:
                            nc.tensor.matmul(
                                cp[0:65, qoff : qoff + w],
                                vaug[kb][:, h * 65 : (h + 1) * 65],
                                es[:, ec : ec + w],
                                start=(state["emitted"] == 0),
                                stop=(state["emitted"] == nkb - 1),
                                skip_group_check=True,
                            )
                            state["emitted"] += 1
                    if state["emitted"] == nkb:
                        if defer_norm:
                            state["norm"] = normalize
                        else:
                            normalize()
                            state["dma"]()

                def normalize():
                    # baseline-proven chain (the custom-DVE reciprocal must
                    # read SBUF, not PSUM). The raw ctx is copied out of PSUM
                    # first so cp frees after ~1.3us instead of after the
                    # whole chain (cp bufs=2 gates the next heads' attention
                    # at small superblocks).
                    cp = state["cp"]
                    d1 = st.tile([1, SQ], f32, name="d1", tag="d1", bufs=1)
                    nc.scalar.copy(d1[:], cp[64:65, :])
                    craw = st.tile([64, SQ], f32, name="craw", tag="craw", bufs=2)
                    nc.vector.tensor_copy(craw[:], cp[0:64, :])
                    rb = st.tile([64, SQ], f32, name="rb", tag="rb", bufs=1)
                    nc.gpsimd.partition_broadcast(rb[:], d1[:])
                    rc = st.tile([64, SQ], f32, name="rc", tag="rc", bufs=1)
                    nc.vector.reciprocal_approx_fast(rc[:], rb[:])
                    nrm = st.tile([64, SQ], bf16, name="nrm", tag="nrm", bufs=1)
                    nc.vector.tensor_mul(nrm[:], craw[:], rc[:])
                    state["nrm"] = nrm

                def norm_dma():
                    nc.sync.dma_start(
                        cfl[par][ti][po : po + 64, :], state["nrm"][:]
                    )

                state["dma"] = norm_dma

                thunks = [
                    (lambda pi=pi: batch(pi)) for pi in range(0, len(pairs), 2)
                ]
                if defer_norm:
                    # compute chain emitted before the tail-fill thunks (the
                    # DVE ops must not queue behind their output casts); only
                    # the cfl DMA goes after them (DMA-queue thresholds are
                    # cumulative for later-emitted instructions)
                    return thunks, (lambda: state["norm"](), norm_dma)
                return thunks

            def make_o_thunks(sb):
                thunks = []
                cfg = cfl[sb & 1]
                for qb in range(4):
                    for n2 in range(2):

                        def group(qb=qb, n2=n2, sb=sb, cfc=cfg):
                            pp = psp.tile([128, SQ], f32, name="pp", tag="pp", bufs=2)
                            for fc in range(4):
                                nc.tensor.matmul(
                                    pp[:],
                                    cfc[fc][:, qb * 128 : (qb + 1) * 128],
                                    wo_sb[fc][:, n2 * SQ : (n2 + 1) * SQ],
                                    start=(fc == 0),
                                    stop=(fc == 3),
                                )
                            ob = st.tile([128, SQ], bf16, name="ob", tag="ob", bufs=2)
                            nc.vector.tensor_copy(ob[:], pp[:])
                            # two half-width DMAs spread over more queues
                            for dh in range(2):
                                nc.sync.dma_start(
                                    out_d[
                                        sb * SQ + qb * 128 : sb * SQ + (qb + 1) * 128,
                                        n2 * SQ + dh * 256 : n2 * SQ + (dh + 1) * 256,
                                    ],
                                    ob[:, dh * 256 : (dh + 1) * 256],
                                )

                        thunks.append(group)
                return thunks

            # ---- emission schedule ----
            # weight DMAs are emitted just before each projection's thunks so
            # the queues deliver them interleaved with that projection's x
            load_weights("k", wk_d)
            thunks0 = make_proj_thunks(0)
            for i, t in enumerate(thunks0):
                if i == 4:
                    load_weights("v", wv_d)
                elif i == 8:
                    load_weights("q", wq_d)
                t()
            load_constants()
            for sb in range(NSC):
                # batch thunks carry a pacing weight: the diagonal (exp/mask
                # heavy, matmul-light) thunks at each head's end get double
                # weight so warm PE work lands right where ScalarE paces
                batches = []
                late_norm = None
                ndiag = 1 if sb % 2 == 0 else 2
                for h in range(HL):
                    if sb == NSC - 1 and h == HL - 1:
                        bts, late_norm = make_attn_batches(h, sb, defer_norm=True)
                    else:
                        bts = make_attn_batches(h, sb)
                    for k, bt in enumerate(bts):
                        batches.append((bt, 2 if k >= len(bts) - ndiag else 1))
                warm = []
                if sb < NSC - 1:
                    warm += make_proj_thunks(sb + 1)
                if sb >= 1:
                    warm += make_o_thunks(sb - 1)
                nw = len(warm)
                tot = sum(wgt for _, wgt in batches)
                # last superblock: hold back some warm o-proj thunks to fill
                # the PE while the final head's normalize chain runs. The
                # deferred normalize is emitted AFTER them so their DMA-queue
                # semaphore thresholds don't include its cfl write.
                hold = 5 if sb == NSC - 1 and nw > 5 else 0
                npaced = nw - hold
                wi = 0
                cum = 0
                for bt, wgt in batches:
                    bt()
                    cum += wgt
                    while wi < npaced and (wi + 1) * tot <= cum * npaced:
                        warm[wi]()
                        wi += 1
                if late_norm is not None:
                    late_norm[0]()
                while wi < nw:
                    warm[wi]()
                    wi += 1
                if late_norm is not None:
                    late_norm[1]()
            for t in make_o_thunks(NSC - 1):
                t()

    nc.compile()
    return nc


def kernel(
    q,
    k,
    v,
    mask=None,
    Wq=None,
    bq=None,
    Wk=None,
    bk=None,
    Wv=None,
    bv=None,
    Wo=None,
    bo=None,
    **_unused,
):
    from concourse.bass_utils import run_bass_kernel_spmd

    if "nc" not in _cache:
        _cache["nc"] = _build()
    nc = _cache["nc"]

    q = np.asarray(q, np.float32)
    k = np.asarray(k, np.float32)
    v = np.asarray(v, np.float32)
    Wq = np.asarray(Wq, np.float32)
    Wk = np.asarray(Wk, np.float32)
    Wv = np.asarray(Wv, np.float32)
    Wo = np.asarray(Wo, np.float32)
    bq = np.zeros(D, np.float32) if bq is None else np.asarray(bq, np.float32)
    bk = np.zeros(D, np.float32) if bk is None else np.asarray(bk, np.float32)
    bv = np.zeros(D, np.float32) if bv is None else np.asarray(bv, np.float32)
    bo = np.zeros(D, np.float32) if bo is None else np.asarray(bo, np.float32)

    import ml_dtypes

    bft = ml_dtypes.bfloat16

    # pair masks (local key k vs es column c):
    # pm0: j0 cols 0:512 (q=c, k<=q); j1 cols 512:896 (q=c-512+128, k<=q-128)
    # pm1: j2 cols 0:256 (q=c+256, k<=q-256); j3 cols 256:384 (q=c+128, k<=q-384)
    kk = np.arange(128)[:, None]
    c0 = np.arange(896)[None, :]
    pm0 = np.where(c0 < 512, kk <= c0, kk <= c0 - 512)
    c1 = np.arange(384)[None, :]
    pm1 = np.where(c1 < 256, kk <= c1, kk <= c1 - 256)
    masks = np.concatenate([pm0, pm1], axis=1).astype(bft)
    zq = np.zeros((64, SQ), bft)

    xT = {}
    for b in range(B):
        xT[("q", b)] = np.ascontiguousarray(q[b].T).astype(bft)
        xT[("k", b)] = np.ascontiguousarray(k[b].T).astype(bft)
        xT[("v", b)] = np.ascontiguousarray(v[b].T).astype(bft)
    wqs, wks, wvs, wos, bqs, bks = {}, {}, {}, {}, {}, {}
    for hg in range(2):
        sl = slice(hg * FH, (hg + 1) * FH)
        wqs[hg] = (np.ascontiguousarray(Wq[sl, :].T) * np.float32(0.125)).astype(bft)
        wks[hg] = np.ascontiguousarray(Wk[sl, :].T).astype(bft)
        wvs[hg] = np.ascontiguousarray(Wv[sl, :].T).astype(bft)
        wos[hg] = np.ascontiguousarray(Wo[:, sl].T).astype(bft)
        bqs[hg] = np.ascontiguousarray(bq[sl]) * np.float32(0.125)
        bks[hg] = np.ascontiguousarray(bk[sl])

    in_maps = []
    for c in range(NCORES):
        b, hg = c // 2, c % 2
        in_maps.append(
            {
                "xq": xT[("q", b)],
                "xk": xT[("k", b)],
                "xv": xT[("v", b)],
                "wq": wqs[hg],
                "wk": wks[hg],
                "wv": wvs[hg],
                "wo": wos[hg],
                "masks": masks,
                "bq": bqs[hg],
                "bk": bks[hg],
                "zq": zq,
            }
        )

    res = run_bass_kernel_spmd(nc, in_maps, list(range(NCORES)))
    out = np.empty((B, S, D), np.float32)
    for b in range(B):
        out[b] = res.results[2 * b]["out"].astype(np.float32) + res.results[
            2 * b + 1
        ]["out"].astype(np.float32)
    const = Wo @ bv + bo  # bv/bo contribution (folds exactly through softmax)
    if np.any(const):
        out += const[None, None, :]
    return out

